# revision 1
# baseline (speedup 1.0000x reference)
"""Multi-head self-attention (B=4, S=2048, D=1024, H=16) on 8 trn2 NeuronCores.

Sharding: data-parallel over (batch, seq-half) -> 8 shards of 1024 query rows.
Each core recomputes K/V for its full batch (k=v=query), so no collectives.

Per-core device kernel (Tile / fp32r matmuls, bf16 for AV + output proj):
  xT = query[b].T (host-transposed, rolled so this core's q rows are cols 0:1023)
  kT[g]  = (Wk x)^T   : [128 dout, 2048 kr]   per head-pair group g (fp32r)
  qT[g]  = (Wq x)^T   : [128 dout, 1024 qr]
  v      = x^T Wv^T   : [kr, dout] natural, +ones column per head (bf16)
  scores^T = kT^T qT  : [kr, qr] per (head, kr-tile)  -> exp via ACT (bf16)
  attn_outT/denom     = v_aug^T exp : [65, qr] PSUM (M=65: 64 hd + denom row)
  normalize via DVE reciprocal + ones-outer-product broadcast, evict bf16
  out = attn_outT^T WoT + (biases all zero) -> [1024 qr, 1024 dout] fp32
"""
import sys

sys.path.insert(0, "/opt/trn_rl_repo")

import numpy as np

B, S, D, H = 4, 2048, 1024, 16
HD = D // H          # 64
P = 128
Q = S // 2           # q rows per core
NG = H // 2          # 8 head-pair groups
DT = D // P          # 8 din tiles
KT = S // P          # 16 kr tiles
SCALE = 1.0 / np.sqrt(np.float32(HD))  # 0.125

_CACHE = {}

# tunables (pool buffer counts)
CFG = {
    "kt_bufs": 3, "qt_bufs": 2, "v_bufs": KT + 1, "exp_bufs": 6,
    "sc_bufs": 2, "av_bufs": 2, "pr_bufs": 2, "stage_bufs": 2,
    "v_inter": True,
}
import os as _os, json as _json
if _os.environ.get("KCFG"):
    CFG.update(_json.loads(_os.environ["KCFG"]))


def _build_bass(repeat=1):
    import concourse.bass as bass
    import concourse.tile as tile
    from concourse import mybir
    from concourse.vector_clock import ScopedClock

    f32 = mybir.dt.float32
    f32r = mybir.dt.float32r
    bf16 = mybir.dt.bfloat16

    # This walrus build only accepts ONE sync-wait per CTRL instruction; the
    # stock Tile exit drain packs all outstanding sem waits onto a single
    # Drain. Spread them across sync-engine nops instead.
    def _drain_and_barrier(self, tick_clock, wait_clock):
        nc = self.nc
        drain_inst = nc.sync.drain()
        wait_clock.add_sem_waits(
            drain_inst.ins, ScopedClock({None: tick_clock.global_clock})
        )
        si = drain_inst.ins.sync_info
        waits = list(si.on_wait) if si is not None else []
        if len(waits) > 1:
            drain_inst.ins.sync_info = mybir.SyncInfo(
                on_wait=waits[:1], on_update=list(si.on_update)
            )
            for i in range(1, len(waits)):
                nop = nc.sync.nop(nofuse=True, hint="drain_wait_split")
                nop.ins.sync_info = mybir.SyncInfo(on_wait=[waits[i]], on_update=[])
        nc.all_engine_barrier()
        popped = nc._tile_sem_poison_stack.pop()
        assert popped is self._sem_poison
        nc.clear_and_free_semaphores(list(self.sems.allocated().values()))
        nc.all_engine_barrier()

    tile.TileContext._drain_and_barrier = _drain_and_barrier

    # Same walrus limitation, general case: any instruction may carry at most
    # one sync wait. Split extras onto same-engine nops placed just before.
    if not getattr(tile.TileContext, "_wait_split_patched", False):
        _orig_lower = tile.TileContext._lower_ordered_insts

        def _lower_with_wait_split(self, ordered):
            counter = 0
            for bb_name in list(ordered.keys()):
                new_insts = []
                for inst in ordered[bb_name]:
                    si = inst.sync_info
                    if si is not None and len(si.on_wait or []) > 1:
                        waits = list(si.on_wait)
                        for w in waits[:-1]:
                            counter += 1
                            nop = mybir.InstNoOp(
                                name=f"I-waitsplit-{bb_name}-{counter}", ins=[], outs=[]
                            )
                            nop.engine = inst.engine
                            nop.sync_info = mybir.SyncInfo(on_wait=[w], on_update=[])
                            new_insts.append(nop)
                        inst.sync_info = mybir.SyncInfo(
                            on_wait=[waits[-1]], on_update=list(si.on_update)
                        )
                    new_insts.append(inst)
                ordered[bb_name] = new_insts
            return _orig_lower(self, ordered)

        tile.TileContext._lower_ordered_insts = _lower_with_wait_split
        tile.TileContext._wait_split_patched = True

    Exp = mybir.ActivationFunctionType.Exp

    nc = bass.Bass()
    # xT/wqT/wkT/wvT arrive pre-rounded to the fp32r grid (host _to_fp32r);
    # woT arrives as bf16
    xT_d = nc.dram_tensor("xT", [D, S], f32r, kind="ExternalInput")
    wqT_d = nc.dram_tensor("wqT", [D, D], f32r, kind="ExternalInput")
    wkT_d = nc.dram_tensor("wkT", [D, D], f32r, kind="ExternalInput")
    wvT_d = nc.dram_tensor("wvT", [D, D], f32r, kind="ExternalInput")
    woT_d = nc.dram_tensor("woT", [D, D], bf16, kind="ExternalInput")
    out_d = nc.dram_tensor("out", [Q, D], f32, kind="ExternalOutput")

    from contextlib import ExitStack

    with tile.TileContext(nc) as tc:
        with ExitStack() as ctx:
            pool = lambda name, bufs, **kw: ctx.enter_context(
                tc.tile_pool(name=name, bufs=bufs, **kw)
            )
            stage_p = None  # inputs arrive pre-converted; no staging needed
            xr_p = pool("xr", DT)            # xT fp32r resident
            wqk_p = pool("wqk", 10)          # per-group W tiles
            wv_p = pool("wv", 9)
            wo_p = pool("wo", 9)
            kt_p = pool("kt", CFG["kt_bufs"])
            qt_p = pool("qt", CFG["qt_bufs"])
            v_p = pool("vv", CFG["v_bufs"])
            exp_p = pool("expp", CFG["exp_bufs"])
            attn_p = pool("attn", DT)
            odd_p = pool("odd", 2)
            rc_p = pool("rc", 2)
            bcb_p = pool("bcb", 2)
            out_p = pool("outp", 3)
            ones_p = pool("ones", 1)
            ps_sc = pool("ps_sc", CFG["sc_bufs"], space="PSUM")
            ps_av = pool("ps_av", CFG["av_bufs"], space="PSUM")
            ps_pr = pool("ps_pr", CFG["pr_bufs"], space="PSUM")
            # ones row for the denominator-broadcast outer product (fp32r via
            # DVE conversion; raw memset bits would not be valid fp32r)
            ones_f = ones_p.tile([P, HD], f32, tag="ones_f")
            nc.vector.memset(ones_f[:], 1.0)
            ones_r = ones_p.tile([P, HD], f32r, tag="ones_r")
            nc.vector.tensor_copy(ones_r[:], ones_f[:])

            for _rep in range(repeat):
                _kernel_body(
                    nc, tc, mybir, f32, f32r, bf16, Exp,
                    xT_d, wqT_d, wkT_d, wvT_d, woT_d, out_d,
                    stage_p, xr_p, wqk_p, wv_p, wo_p, kt_p, qt_p, v_p, exp_p,
                    attn_p, odd_p, rc_p, bcb_p, out_p, ones_r,
                    ps_sc, ps_av, ps_pr,
                )
    return nc


def _kernel_body(
    nc, tc, mybir, f32, f32r, bf16, Exp,
    xT_d, wqT_d, wkT_d, wvT_d, woT_d, out_d,
    stage_p, xr_p, wqk_p, wv_p, wo_p, kt_p, qt_p, v_p, exp_p,
    attn_p, odd_p, rc_p, bcb_p, out_p, ones_r,
    ps_sc, ps_av, ps_pr,
):
    if True:
        if True:
            # load xT (pre-rounded fp32r bits) straight into resident tiles.
            # Chunked along kr so the first projection psum group can start
            # as soon as its slice of every din tile has landed.
            xr = [xr_p.tile([P, S], f32r, tag="xr", name=f"xr{dt}") for dt in range(DT)]
            for ch in range(4):  # chunk-major: kr chunk 0 of every tile first
                for dt in range(DT):
                    nc.sync.dma_start(
                        xr[dt][:, ch * 512:(ch + 1) * 512],
                        xT_d[dt * P:(dt + 1) * P, ch * 512:(ch + 1) * 512],
                    )

            attn_sb = []
            v_sb = [None] * KT
            for g in range(NG):
                # ---- W tiles for this group's dout columns ----
                wq_r, wk_r = [], []
                for dt in range(DT):
                    for w_d, lst, tg in ((wqT_d, wq_r, "wq"), (wkT_d, wk_r, "wk")):
                        wr = wqk_p.tile([P, P], f32r, tag=tg)
                        nc.sync.dma_start(
                            wr[:], w_d[dt * P:(dt + 1) * P, g * P:(g + 1) * P]
                        )
                        lst.append(wr)

                # ---- qT projection first: [128 dout, 1024 qr] ----
                # (QK t=0 needs qt chunk 0 + kt chunk 0; qT-first unblocks the
                # first attention matmuls ~16 MMs earlier at group starts)
                qt = qt_p.tile([P, Q], f32r, tag="qt")
                for n in range(2):
                    ps = ps_pr.tile([P, 512], f32, tag="prj")
                    for dt in range(DT):
                        nc.tensor.matmul(
                            ps[:], wq_r[dt][:], xr[dt][:, n * 512:(n + 1) * 512],
                            start=(dt == 0), stop=(dt == DT - 1),
                        )
                    nc.vector.tensor_copy(qt[:, n * 512:(n + 1) * 512], ps[:])

                # ---- kT projection: [128 dout, 2048 kr] ----
                kt = kt_p.tile([P, S], f32r, tag="kt")
                for n in range(4):
                    ps = ps_pr.tile([P, 512], f32, tag="prj")
                    for dt in range(DT):
                        nc.tensor.matmul(
                            ps[:], wk_r[dt][:], xr[dt][:, n * 512:(n + 1) * 512],
                            start=(dt == 0), stop=(dt == DT - 1),
                        )
                    nc.vector.tensor_copy(kt[:, n * 512:(n + 1) * 512], ps[:])

                # ---- V projection for superblock of 8 heads (every 4 groups) ----
                vproj = g % 4 == 0
                if vproj:
                    sbi = g // 4
                    wv_r = []
                    for dt in range(DT):
                        wr = wv_p.tile([P, 512], f32r, tag="wv")
                        nc.sync.dma_start(
                            wr[:],
                            wvT_d[dt * P:(dt + 1) * P, sbi * 512:(sbi + 1) * 512],
                        )
                        wv_r.append(wr)

                def _project_v(t):
                    ps = ps_pr.tile([P, 512], f32, tag="prj", name=f"vps{g}_{t}")
                    for dt in range(DT):
                        nc.tensor.matmul(
                            ps[:], xr[dt][:, t * P:(t + 1) * P], wv_r[dt][:],
                            start=(dt == 0), stop=(dt == DT - 1),
                        )
                    vt = v_p.tile([P, 8 * (HD + 1)], bf16, tag="v", name=f"v{g}_{t}")
                    # v columns: 8 head-slots of [64 v | 1 one]
                    dst = vt[:].rearrange("p (h c) -> p h c", c=HD + 1)
                    src = ps[:].rearrange("p (h c) -> p h c", c=HD)
                    nc.vector.tensor_copy(dst[:, :, 0:HD], src[:])
                    nc.vector.memset(dst[:, :, HD:HD + 1], 1.0)
                    v_sb[t] = vt

                if vproj and not CFG["v_inter"]:
                    for t in range(KT):
                        _project_v(t)

                # ---- attention for heads 2g, 2g+1, interleaved so the two
                # heads' K=64 QK matmuls pack onto disjoint PE row groups ----
                at = attn_p.tile([P, Q], bf16, tag="attn")
                for qr in range(2):
                    q0 = qr * 512
                    av = [
                        ps_av.tile([P, 512], f32, tag="av", name=f"av{g}_{qr}_{i}")
                        for i in range(2)
                    ]
                    for t in range(KT):
                        if vproj and qr == 0 and CFG["v_inter"]:
                            # project v for this kr-tile just-in-time so the
                            # attention stream starts without waiting for the
                            # whole superblock's V projection
                            _project_v(t)
                        sc = ps_sc.tile([P, Q], f32, tag="sc")
                        for h_loc in range(2):
                            r0 = h_loc * HD
                            nc.tensor.matmul(
                                sc[:, h_loc * 512:(h_loc + 1) * 512],
                                kt[r0:r0 + HD, t * P:(t + 1) * P],
                                qt[r0:r0 + HD, q0:q0 + 512],
                                start=True, stop=True,
                            )
                        ex = exp_p.tile([P, Q], bf16, tag="ex")
                        nc.scalar.activation(ex[:], sc[:], Exp, scale=float(SCALE))
                        for h_loc in range(2):
                            hs = ((2 * g + h_loc) % 8) * (HD + 1)
                            nc.tensor.matmul(
                                av[h_loc][0:HD + 1, :],
                                v_sb[t][:, hs:hs + HD + 1],
                                ex[:, h_loc * 512:(h_loc + 1) * 512],
                                start=(t == 0), stop=(t == KT - 1),
                            )
                    # normalize by the denominator row (partition HD=64)
                    for h_loc in range(2):
                        rcp = rc_p.tile([P, 512], f32r, tag="rc")
                        with nc.allow_low_precision(reason="fp32r recip of softmax denom"):
                            nc.vector.reciprocal(
                                rcp[HD:HD + 1, :], av[h_loc][HD:HD + 1, :]
                            )
                        bc = ps_pr.tile([P, 512], f32, tag="prj")
                        nc.tensor.matmul(
                            bc[0:HD, :], ones_r[HD:HD + 1, 0:HD], rcp[HD:HD + 1, :],
                            start=True, stop=True,
                        )
                        # TT may read at most one PSUM operand: bounce the
                        # broadcast through SBUF
                        bcs = bcb_p.tile([HD, 512], f32, tag="bcb")
                        nc.vector.tensor_copy(bcs[:], bc[0:HD, :])
                        if h_loc == 0:
                            nc.vector.tensor_mul(
                                at[0:HD, q0:q0 + 512], av[h_loc][0:HD, :], bcs[:]
                            )
                        else:
                            odd_t = odd_p.tile([HD, 512], bf16, tag="odd")
                            nc.vector.tensor_mul(
                                odd_t[:], av[h_loc][0:HD, :], bcs[:]
                            )
                            # odd head's rows belong at partitions 64:128
                            nc.sync.dma_start(at[HD:P, q0:q0 + 512], odd_t[:])
                attn_sb.append(at)
                if g == NG - 2:
                    # prefetch the first Wo weight batch during the
                    # second-to-last group's attention so the output
                    # projection starts with weights already resident
                    wo_pre = []
                    for dt in range(DT):
                        wr = wo_p.tile([P, 512], bf16, tag="wo", name=f"wopre{dt}")
                        nc.sync.dma_start(wr[:], woT_d[dt * P:(dt + 1) * P, 0:512])
                        wo_pre.append(wr)

            # ---- output projection: out[qr, dout] ----
            for nh in range(2):
                if nh == 0:
                    wo_r = wo_pre
                else:
                    wo_r = []
                    for dt in range(DT):
                        wr = wo_p.tile([P, 512], bf16, tag="wo")
                        nc.sync.dma_start(
                            wr[:], woT_d[dt * P:(dt + 1) * P, nh * 512:(nh + 1) * 512]
                        )
                        wo_r.append(wr)
                for qrt in range(Q // P):
                    ps = ps_pr.tile([P, 512], f32, tag="prj")
                    for dt in range(DT):
                        nc.tensor.matmul(
                            ps[:], attn_sb[dt][:, qrt * P:(qrt + 1) * P], wo_r[dt][:],
                            start=(dt == 0), stop=(dt == DT - 1),
                        )
                    ot = out_p.tile([P, 512], f32, tag="out")
                    nc.vector.tensor_copy(ot[:], ps[:])
                    nc.sync.dma_start(
                        out_d[qrt * P:(qrt + 1) * P, nh * 512:(nh + 1) * 512], ot[:]
                    )
    return nc


def _get_exec(repeat=1):
    """Build the Bass module once and wrap it in a cached 8-core jitted callable."""
    key = ("exec", repeat)
    if key in _CACHE:
        return _CACHE[key]

    import jax
    import concourse.mybir as mybir
    from concourse import bass2jax
    from jax.experimental.shard_map import shard_map
    from jax.sharding import Mesh, PartitionSpec

    nc = _build_bass(repeat)
    bass2jax.install_neuronx_cc_hook()

    partition_name = nc.partition_id_tensor.name if nc.partition_id_tensor else None
    in_names, out_names, out_avals = [], [], []
    for alloc in nc.m.functions[0].allocations:
        if not isinstance(alloc, mybir.MemoryLocationSet):
            continue
        name = alloc.memorylocations[0].name
        if alloc.kind == "ExternalInput":
            if name != partition_name:
                in_names.append(name)
        elif alloc.kind == "ExternalOutput":
            out_names.append(name)
            out_avals.append(
                jax.core.ShapedArray(tuple(alloc.tensor_shape), mybir.dt.np(alloc.dtype))
            )
    n_params = len(in_names)
    all_names = in_names + out_names
    if partition_name is not None:
        all_names = all_names + [partition_name]

    def _body(*args):
        operands = list(args)
        if partition_name is not None:
            operands.append(bass2jax.partition_id_tensor())
        outs = bass2jax._bass_exec_p.bind(
            *operands,
            out_avals=tuple(out_avals),
            in_names=tuple(all_names),
            out_names=tuple(out_names),
            lowering_input_output_aliases=(),
            sim_require_finite=True,
            sim_require_nnan=True,
            nc=nc,
        )
        return tuple(outs)

    devices = jax.devices()[:8]
    mesh = Mesh(np.asarray(devices), ("core",))
    n_out = len(out_names)
    sharded = jax.jit(
        shard_map(
            _body,
            mesh=mesh,
            in_specs=(PartitionSpec("core"),) * (n_params + n_out),
            out_specs=(PartitionSpec("core"),) * n_out,
            check_rep=False,
        ),
        keep_unused=True,
    )
    _CACHE[("nc", repeat)] = nc
    _CACHE["nc"] = nc
    _CACHE[key] = (sharded, in_names, out_names, out_avals)
    return _CACHE[key]


def _chained_exec(k_iters):
    """Jitted fn running the NEFF k times back-to-back (serialized via data
    deps) on all 8 cores. Used to measure marginal per-iteration HW time."""
    import jax
    import jax.numpy as jnp
    import concourse.mybir as mybir
    from concourse import bass2jax
    from jax.experimental.shard_map import shard_map
    from jax.sharding import Mesh, PartitionSpec

    sharded, in_names, out_names, out_avals = _get_exec()  # ensures nc built
    nc = _CACHE["nc"]
    partition_name = nc.partition_id_tensor.name if nc.partition_id_tensor else None
    all_names = in_names + out_names
    if partition_name is not None:
        all_names = all_names + [partition_name]
    n_params = len(in_names)
    xT_idx = in_names.index("xT")

    def _body(*args):
        ins = list(args[:n_params])
        zeros = list(args[n_params:])
        out = None
        for _ in range(k_iters):
            operands = list(ins) + list(zeros)
            if partition_name is not None:
                operands.append(bass2jax.partition_id_tensor())
            outs = bass2jax._bass_exec_p.bind(
                *operands,
                out_avals=tuple(out_avals),
                in_names=tuple(all_names),
                out_names=tuple(out_names),
                lowering_input_output_aliases=(),
                sim_require_finite=True,
                sim_require_nnan=True,
                nc=nc,
            )
            out = outs[0]
            # serialize iterations: next xT depends on this out
            ins[xT_idx] = jnp.concatenate([out, out], axis=1)
        return (out,)

    devices = jax.devices()[:8]
    mesh = Mesh(np.asarray(devices), ("core",))
    n_out = len(out_names)
    return jax.jit(
        shard_map(
            _body,
            mesh=mesh,
            in_specs=(PartitionSpec("core"),) * (n_params + n_out),
            out_specs=(PartitionSpec("core"),) * n_out,
            check_rep=False,
        ),
        keep_unused=True,
    )


def _to_fp32r(a):
    """Round fp32 to the fp32r grid: RNE at the low-12-mantissa-bit boundary
    (matches walrus fp32_to_fp32r: downconv to e8m11, stored <<12)."""
    u = np.ascontiguousarray(a, np.float32).view(np.uint32)
    low = u & np.uint32(0xFFF)
    base = u & ~np.uint32(0xFFF)
    round_up = (low > 0x800) | ((low == 0x800) & (((u >> 12) & 1) == 1))
    return (base + (round_up.astype(np.uint32) << 12)).view(np.float32)


def _prep_in_maps(query, WqT, WkT, WvT, WoT):
    import ml_dtypes

    WqTr, WkTr, WvTr = _to_fp32r(WqT), _to_fp32r(WkT), _to_fp32r(WvT)
    WoTb = np.ascontiguousarray(WoT).astype(ml_dtypes.bfloat16)
    in_maps = []
    for c in range(8):
        b, half = c // 2, c % 2
        xT = query[b].T
        if half == 1:
            xT = np.concatenate([xT[:, Q:], xT[:, :Q]], axis=1)
        in_maps.append({
            "xT": _to_fp32r(xT),
            "wqT": WqTr, "wkT": WkTr, "wvT": WvTr, "woT": WoTb,
        })
    return in_maps


def _run_device(in_maps):
    sharded, in_names, out_names, out_avals = _get_exec()
    concat_in = [
        np.concatenate([m[name] for m in in_maps], axis=0) for name in in_names
    ]
    zeros = [
        np.zeros((8 * a.shape[0], *a.shape[1:]), a.dtype) for a in out_avals
    ]
    out_arrs = sharded(*concat_in, *zeros)
    per_core = []
    for c in range(8):
        per_core.append({
            name: np.asarray(out_arrs[i]).reshape(8, *out_avals[i].shape)[c]
            for i, name in enumerate(out_names)
        })
    return per_core


def _numpy_fallback(query, Wq, bq, Wk, bk, Wv, bv, Wo, bo):
    q = query @ Wq.T + bq
    k = query @ Wk.T + bk
    v = query @ Wv.T + bv
    q = q.reshape(B, S, H, HD).transpose(0, 2, 1, 3)
    k = k.reshape(B, S, H, HD).transpose(0, 2, 1, 3)
    v = v.reshape(B, S, H, HD).transpose(0, 2, 1, 3)
    scores = np.einsum("bhqd,bhkd->bhqk", q, k) / np.sqrt(np.float32(HD))
    scores -= scores.max(axis=-1, keepdims=True)
    e = np.exp(scores)
    attn = e / e.sum(axis=-1, keepdims=True)
    out = np.einsum("bhqk,bhkd->bhqd", attn, v)
    out = out.transpose(0, 2, 1, 3).reshape(B, S, D)
    return (out @ Wo.T + bo).astype(np.float32)


def kernel(query, Wq, bq, Wk, bk, Wv, bv, Wo, bo):
    query = np.asarray(query, np.float32)
    Wq, Wk, Wv, Wo = (np.asarray(w, np.float32) for w in (Wq, Wk, Wv, Wo))
    bq, bk, bv, bo = (np.asarray(b_, np.float32) for b_ in (bq, bk, bv, bo))
    if any(np.any(b_) for b_ in (bq, bk, bv, bo)):
        return _numpy_fallback(query, Wq, bq, Wk, bk, Wv, bv, Wo, bo)

    WqT = np.ascontiguousarray(Wq.T)
    WkT = np.ascontiguousarray(Wk.T)
    WvT = np.ascontiguousarray(Wv.T)
    WoT = np.ascontiguousarray(Wo.T)
    in_maps = _prep_in_maps(query, WqT, WkT, WvT, WoT)
    per_core = _run_device(in_maps)
    out = np.empty((B, S, D), np.float32)
    for c in range(8):
        b, half = c // 2, c % 2
        out[b, half * Q:(half + 1) * Q] = per_core[c]["out"]
    return out

